# revision 21
# baseline (speedup 1.0000x reference)
"""Trainium2 Bass kernel for nn_CrossAttention (B=8, S1=S2=2048, D=512, single head).

Sharding: batch dim B=8 across the 8 NeuronCores (data parallel). Each core runs
the full cross-attention for one batch element:
    q = RoPE(h1 @ Wq.T + bq); k = RoPE(h2 @ Wk.T + bk); v = h2 @ Wv.T + bv
    out = softmax(q k^T / sqrt(D)) v @ Wo.T + bo

Design notes (v4):
  - All matmuls in bf16 (fp32 PSUM accumulation): rel_l2 vs fp32 reference ~6e-3.
  - Scores are computed TRANSPOSED (S^T[k,q]) so the probability matrix feeds the
    PV matmul directly as the moving operand - no P transposes.
  - Softmax skips max-subtraction (energies are ~N(0,1), exp is safe in fp32).
  - Colsums (denominators) via an ALL-ONES [128,128] stationary matmul: a
    1-column ones vector gets a col_grp LDWEIGHTS that can't overlap the
    neighbouring matmuls (~190ns/kb); the full-width version loads via FWL at
    full overlap and lands the colsum REPLICATED across partitions.  A tiny
    basis-vector matmul (cs_block @ e0) then moves the colsums onto partitions
    with no PE transposes; reciprocal runs wide on [128,4].
  - Attention kb pipeline runs colsum/PV at lag TWO behind S^T so exp(kb) is
    long done when PV(kb) issues.
  - DMA completions coalesce onto ONE counting semaphore: every consumer
    emitted after a dma_start waits for ALL earlier-emitted DMAs.  So DMAs are
    emitted in exact first-use order, late tensors (wo/bo) issue at the END of
    phase A, and the DMA count is minimized (fused bk+bq, fused h1 slices 1-3,
    one output DMA per q tile) - fewer DMAs also shrink the fixed semaphore-
    zeroing teardown (~150ns per DMA per queue).
  - Phase A order k0 q0 k1 k2 v0 v1 k3 v2 v3 matches the ~350GB/s DMA front:
    by the time the PE needs wv (v0) the transfer has landed.
  - q slices 1-3 project inside the attention kb loop (pair chunks at kb==1 and
    kb==6) in the fin PSUM slot.
  - PV pass 2 accumulates into st-pool banks; the four final-projection outputs
    alternate between the ot and fin slots (both free by then) because tile
    WAR tracking is tile-granular: distinct tiles keep one slice's normalize
    read from stalling the next slice's matmuls.
"""

import math
import sys

import numpy as np

for _p in ("/opt/trn_rl_repo",):
    if _p not in sys.path:
        sys.path.insert(0, _p)

import ml_dtypes

BF16 = ml_dtypes.bfloat16

S = 2048
D = 512
P = 128
B = 8
NB = S // P      # 16 key blocks of 128
DC = D // P      # 4 d-chunks of 128
EC = D // P      # 4 e-chunks (contraction for projections)
QW = 512         # tile width (free dim per matmul)
QT = S // QW     # 4 q tiles
SB = QW // P     # 4 s-blocks per q tile
NS = S // QW     # 4 s-slices for the prologue
SCALE = 1.0 / math.sqrt(D)

_compiled = None


def _build():
    import concourse.bass as bass  # noqa: F401
    import concourse.mybir as mybir
    import concourse.tile as tile
    from concourse import bacc

    f32 = mybir.dt.float32
    bf16 = mybir.dt.bfloat16
    Alu = mybir.AluOpType
    Act = mybir.ActivationFunctionType

    nc = bacc.Bacc("TRN2", target_bir_lowering=False, debug=False, num_devices=B)

    # All large inputs arrive packed in their exact per-partition SBUF layout
    # (host does transpose/cast/shuffle): each partition's data is one
    # contiguous run, so DMAs use maximum-size packets on a single queue.
    # h1t/h2t: h^T as [p, (s2 ec sq)]; weights: W^T as [p, (ec d)]; tabs holds
    # cos/sin both pairs slice-major: [p, (s2 cs pair sq)] (tables are
    # half-size because emb = concat([freqs, freqs])).
    h1t_d = nc.dram_tensor("h1t", [P, NS, EC, QW], bf16, kind="ExternalInput").ap()
    h2t_d = nc.dram_tensor("h2t", [P, NS, EC, QW], bf16, kind="ExternalInput").ap()
    w_dram = {
        name: nc.dram_tensor(f"{name}_t", [P, EC * D], bf16, kind="ExternalInput").ap()
        for name in ("wq", "wk", "wv", "wo")
    }
    tabs_d = nc.dram_tensor("tabs", [P, NS, 2, 2, QW], bf16, kind="ExternalInput").ap()
    # bkq packs bk (c=0) and bq (c=1); bo_b holds bo_eff = bo + Wo @ bv
    bkq_c = nc.dram_tensor("bkq_c", [P, 2, DC], f32, kind="ExternalInput").ap()
    bo_b = nc.dram_tensor("bo_b", [P, D], f32, kind="ExternalInput").ap()
    out = nc.dram_tensor("out", [S, D], f32, kind="ExternalOutput").ap()
    out_r = out.rearrange("(qt sb p) d -> qt p sb d", p=P, sb=SB)

    with tile.TileContext(nc) as tc:
        from contextlib import ExitStack

        with ExitStack() as ctx:
            singles = ctx.enter_context(tc.tile_pool(name="singles", bufs=1))
            scratch = ctx.enter_context(tc.tile_pool(name="scratch", bufs=3))

            def load_w(name, eng):
                # one dma_start per weight: DMA *issue* costs ~0.7us on the
                # sequencer, so fewer+bigger transfers win at the front
                t = singles.tile([P, EC, D], bf16, tag=f"w_{name}")
                eng.dma_start(
                    out=t, in_=w_dram[name].rearrange("p (c d) -> p c d", d=D)
                )
                return t

            # --- persistent tiles -------------------------------------------
            w_sb = {}
            kt_p = [
                singles.tile([P, DC, QW], bf16, tag=f"kt{i}", name=f"kt{i}")
                for i in range(NS)
            ]
            qt_p = [
                singles.tile([P, DC, QW], bf16, tag=f"qt{i}", name=f"qt{i}")
                for i in range(NS)
            ]
            v_p = [
                singles.tile([P, SB, QW], bf16, tag=f"v{i}", name=f"v{i}")
                for i in range(NS)
            ]
            h1s0 = singles.tile([P, EC, QW], bf16, tag="h1s0", name="h1s0")
            h1sr = singles.tile([P, NS - 1, EC, QW], bf16, tag="h1sr", name="h1sr")
            h1s = [h1s0] + [h1sr[:, i] for i in range(NS - 1)]
            h2s = [
                singles.tile([P, EC, QW], bf16, tag=f"h2s{i}", name=f"h2s{i}")
                for i in range(NS)
            ]
            tabs_sb = singles.tile([P, NS, 2, 2, QW], bf16, tag="tabs")

            # --- DMA emission striped across the 3 queues in NEED order -----
            # only sync/scalar/gpsimd can issue DMAs; each queue serializes its
            # own transfers and the ~350GB/s aggregate is shared (~115GB/s per
            # active queue), so the global need-order must round-robin across
            # queues or an early queue-mate delays a critical transfer by 4us+
            # each queue's K-th transfer lands at ~K*4.5us (aggregate shared
            # ~3 ways), so the critical tensors take the EARLY slots of each
            # queue; gpsimd's slot 2 is nearly free (tiny bkq), making its
            # slots 3-5 the right home for the later h2 slices
            # gpsimd's DMA path is software-dynamic (slow) - big transfers
            # ride the two hardware queues (sync, scalar) only, ordered by
            # first use; q0 projects LATE in phase A so h1s0/wq vacate the
            # early queue slots for the k-slice/v-path tensors
            nc.sync.dma_start(out=h2s[0], in_=h2t_d[:, 0])
            w_sb["wk"] = load_w("wk", nc.scalar)
            nc.gpsimd.dma_start(out=tabs_sb[:, 0], in_=tabs_d[:, 0])
            nc.sync.dma_start(out=h2s[1], in_=h2t_d[:, 1])
            w_sb["wv"] = load_w("wv", nc.scalar)
            bkq_sb = singles.tile([P, 2, DC], f32, tag="bkq")
            nc.gpsimd.dma_start(out=bkq_sb, in_=bkq_c)
            bk_sb = bkq_sb[:, 0]
            bq_sb = bkq_sb[:, 1]
            nc.sync.dma_start(out=h2s[2], in_=h2t_d[:, 2])
            nc.scalar.dma_start(out=tabs_sb[:, 1], in_=tabs_d[:, 1])
            nc.sync.dma_start(out=h2s[3], in_=h2t_d[:, 3])
            w_sb["wq"] = load_w("wq", nc.scalar)
            nc.sync.dma_start(out=h1s0, in_=h1t_d[:, 0])
            nc.scalar.dma_start(out=tabs_sb[:, 2], in_=tabs_d[:, 2])
            nc.sync.dma_start(out=tabs_sb[:, 3], in_=tabs_d[:, 3])
            # all-ones stationary for colsums + basis vector e0 for the
            # denominator extraction
            ones128 = singles.tile([P, P], bf16, tag="ones128")
            nc.vector.memset(ones128, 1.0)
            e0 = singles.tile([P, 1], bf16, tag="e0")
            nc.vector.memset(e0, 0.0)
            nc.vector.memset(e0[0:1, :], 1.0)

            def emit_proj_pair(ht, wname, b_sb, dst, s2, pair, pool, tag):
                # dst[:, {pair, pair+2}, :] = RoPE(W @ h^T + b) for slice s2
                dc0, dc2 = pair, pair + 2
                pp = pool.tile([P, 2, QW], f32, tag=tag, name="pp")
                for half, dc in ((0, dc0), (1, dc2)):
                    for ec in range(EC):
                        nc.tensor.matmul(
                            pp[:, half, :],
                            lhsT=w_sb[wname][:, ec, dc * P : (dc + 1) * P],
                            rhs=ht[:, ec, :],
                            start=(ec == 0),
                            stop=(ec == EC - 1),
                        )
                # rope: out[d<256] = x0*cos - x2*sin ; out[d>=256] = x2*cos + x0*sin
                # (bias folds into the STT's scalar add; the combines run on the
                # otherwise-idle GpSimd engine)
                cps = tabs_sb[:, s2, 0, pair, :]
                sps = tabs_sb[:, s2, 1, pair, :]
                t0 = scratch.tile([P, QW], f32, tag="rope0", name="t0")
                nc.vector.scalar_tensor_tensor(
                    t0, in0=pp[:, 0, :], scalar=b_sb[:, dc0 : dc0 + 1], in1=cps,
                    op0=Alu.add, op1=Alu.mult,
                )
                t1 = scratch.tile([P, QW], f32, tag="rope1", name="t1")
                nc.vector.scalar_tensor_tensor(
                    t1, in0=pp[:, 1, :], scalar=b_sb[:, dc2 : dc2 + 1], in1=sps,
                    op0=Alu.add, op1=Alu.mult,
                )
                nc.gpsimd.tensor_tensor(dst[:, dc0, :], t0, t1, Alu.subtract)
                t2 = scratch.tile([P, QW], f32, tag="rope0", name="t2")
                nc.vector.scalar_tensor_tensor(
                    t2, in0=pp[:, 1, :], scalar=b_sb[:, dc2 : dc2 + 1], in1=cps,
                    op0=Alu.add, op1=Alu.mult,
                )
                t3 = scratch.tile([P, QW], f32, tag="rope1", name="t3")
                nc.vector.scalar_tensor_tensor(
                    t3, in0=pp[:, 0, :], scalar=b_sb[:, dc0 : dc0 + 1], in1=sps,
                    op0=Alu.add, op1=Alu.mult,
                )
                nc.gpsimd.tensor_tensor(dst[:, dc2, :], t2, t3, Alu.add)

            def project_v(s2, psV):
                # bv is folded into bo on host (bo_eff = bo + Wo @ bv), so this
                # is a plain PSUM->SBUF cast on the idle ACT engine
                for j in range(SB):
                    vp = psV.tile([P, QW], f32, tag="vp", bufs=2, name="vp")
                    for ec in range(EC):
                        nc.tensor.matmul(
                            vp,
                            lhsT=h2s[s2][:, ec, j * P : (j + 1) * P],
                            rhs=w_sb["wv"][:, ec, :],
                            start=(ec == 0),
                            stop=(ec == EC - 1),
                        )
                    nc.scalar.copy(v_p[s2][:, j, :], vp)

            # ---------------- Phase A: k/v (+ q0) projections + RoPE --------
            # emission order k0 q0 k1 k2 v0 v1 k3 v2 v3 tracks the DMA front:
            # wv's transfer lands right as the PE reaches v0
            with tc.tile_pool(name="psA", bufs=3, space="PSUM") as psA:
                def proj_k(s2):
                    for pair in range(2):
                        emit_proj_pair(h2s[s2], "wk", bk_sb, kt_p[s2], s2, pair, psA, "pp")

                proj_k(0)
                proj_k(1)
                proj_k(2)
                project_v(0, psA)
                project_v(1, psA)
                proj_k(3)
                for pair in range(2):
                    emit_proj_pair(h1s[0], "wq", bq_sb, qt_p[0], 0, pair, psA, "pp")
                project_v(2, psA)
                project_v(3, psA)
                # late-needed tensors issue LAST (single DMA-completion counter:
                # anything emitted after an early wo/bo would wait on them)
                nc.sync.dma_start(out=h1sr, in_=h1t_d[:, 1:NS])
                w_sb["wo"] = load_w("wo", nc.scalar)
                bo_sb = singles.tile([P, D], f32, tag="bo")
                nc.gpsimd.dma_start(out=bo_sb, in_=bo_b)

            # ---------------- Phase B: attention -----------------------------
            # PSUM budget (8 banks) in one pool: st 3 + ot 2 + cs 1 + fin 2.
            with tc.tile_pool(name="psB", bufs=1, space="PSUM") as psB:
                for qt in range(QT):
                    cs = psB.tile([P, QW], f32, tag="cs", name=f"cs{qt}")
                    ot_sb = scratch.tile([P, DC, QW], bf16, tag="ot_sb", bufs=2, name="ot_sb")
                    pts = []

                    # pass 1: S^T + exp + colsums + PV dc 0,1 at lag 2: exp(kb)
                    # is always finished before PV(kb) issues on the PE
                    ot01 = psB.tile([P, 2, QW], f32, tag="ot", name="ot01")
                    for kb in range(NB):
                        st = psB.tile([P, QW], f32, tag="st", bufs=3, name="st")
                        for dc in range(DC):
                            nc.tensor.matmul(
                                st,
                                lhsT=kt_p[kb // SB][:, dc, (kb % SB) * P : (kb % SB + 1) * P],
                                rhs=qt_p[qt][:, dc, :],
                                start=(dc == 0),
                                stop=(dc == DC - 1),
                            )
                        pt = scratch.tile([P, QW], bf16, tag="pt", bufs=NB + 2, name="pt")
                        nc.scalar.activation(pt, st, Act.Exp, scale=SCALE)
                        pts.append(pt)
                        if kb >= 2:
                            _emit_pv(nc, v_p, ones128, pts[kb - 2], ot01, cs, kb - 2, (0, 1))
                        if qt + 1 < QT and kb == 2:
                            # project+RoPE the next q slice inside this q tile's
                            # attention stream, one pair chunk at a time
                            emit_proj_pair(
                                h1s[qt + 1], "wq", bq_sb, qt_p[qt + 1], qt + 1, 0,
                                psB, "fin",
                            )
                        if qt + 1 < QT and kb == 8:
                            emit_proj_pair(
                                h1s[qt + 1], "wq", bq_sb, qt_p[qt + 1], qt + 1, 1,
                                psB, "fin",
                            )
                    _emit_pv(nc, v_p, ones128, pts[NB - 2], ot01, cs, NB - 2, (0, 1))
                    _emit_pv(nc, v_p, ones128, pts[NB - 1], ot01, cs, NB - 1, (0, 1))

                    # ot01 -> SBUF on two engines in parallel; the replicated
                    # colsum block copies (bf16) for the basis matmuls
                    nc.scalar.copy(ot_sb[:, 0, :], ot01[:, 0, :])
                    nc.vector.tensor_copy(out=ot_sb[:, 1, :], in_=ot01[:, 1, :])
                    cs_sb = scratch.tile([P, QW], bf16, tag="cs_sb", bufs=2, name="cs_sb")
                    nc.scalar.copy(cs_sb, cs)

                    # pass 2: PV dc 2,3 from the resident pt tiles, accumulated
                    # into st-pool banks (free after the kb loop)
                    otA = psB.tile([P, QW], f32, tag="st", bufs=3, name=f"otA{qt}")
                    otB = psB.tile([P, QW], f32, tag="st", bufs=3, name=f"otB{qt}")
                    for kb in range(NB):
                        for ot2, dc in ((otA, 2), (otB, 3)):
                            nc.tensor.matmul(
                                ot2,
                                lhsT=v_p[kb // SB][:, kb % SB, dc * P : (dc + 1) * P],
                                rhs=pts[kb],
                                start=(kb == 0),
                                stop=(kb == NB - 1),
                            )
                        if kb == 8:
                            # denominators: the colsum is REPLICATED across
                            # partitions, so block @ e0 lands colsum[sb*128+p]
                            # on partition p - no transposes, full-width LDs
                            r4ps = psB.tile([P, SB], f32, tag="cs", name=f"r4ps{qt}")
                            for sb in range(SB):
                                nc.tensor.matmul(
                                    r4ps[:, sb : sb + 1],
                                    lhsT=cs_sb[:, sb * P : (sb + 1) * P],
                                    rhs=e0,
                                    start=True,
                                    stop=True,
                                )
                            # reciprocal runs DURING pass 2 so the DVE is clear
                            # for the normalize chain at the q-tile boundary
                            r4r = scratch.tile([P, SB], f32, tag="r4r", bufs=2, name="r4r")
                            nc.vector.reciprocal(r4r, r4ps)
                    # otA on ACT, otB on DVE... both PSUM reads off the PE path
                    nc.scalar.copy(ot_sb[:, 2, :], otA)
                    nc.vector.tensor_copy(out=ot_sb[:, 3, :], in_=otB)

                    # final projection back to natural [s, d] layout; the four
                    # output PSUM tiles alternate between the ot and fin slots
                    # (both free by now) - WAR tracking is tile-granular, so
                    # distinct tiles keep each slice's normalize read from
                    # stalling the next slice's matmuls
                    o_q = scratch.tile([P, SB, D], f32, tag="ostage", bufs=2, name="o_q")
                    for sb in range(SB):
                        fpt = psB.tile(
                            [P, 2, QW], f32,
                            tag=("ot" if sb % 2 == 0 else "fin"),
                            name=f"fp{qt}_{sb}",
                        )
                        fp = fpt[:, 0, :]
                        for dc in range(DC):
                            nc.tensor.matmul(
                                fp,
                                lhsT=ot_sb[:, dc, sb * P : (sb + 1) * P],
                                rhs=w_sb["wo"][:, dc, :],
                                start=(dc == 0),
                                stop=(dc == DC - 1),
                            )
                        if qt == QT - 1 and sb % 2 == 1:
                            # last q tile: odd slices normalize via ACT (PSUM
                            # scale-read) + GpSimd (SBUF bias add) - ACT is
                            # idle here and the DVE chain stops gating the tail
                            o_t = scratch.tile([P, D], f32, tag="oscale", bufs=2, name="o_t")
                            nc.scalar.activation(
                                o_t, fp, Act.Copy, scale=r4r[:, sb : sb + 1]
                            )
                            nc.gpsimd.tensor_tensor(o_q[:, sb, :], o_t, bo_sb, Alu.add)
                        else:
                            # fused (fp * r) + bo in one DVE op (DVE reads PSUM)
                            nc.vector.scalar_tensor_tensor(
                                o_q[:, sb, :],
                                in0=fp,
                                scalar=r4r[:, sb : sb + 1],
                                in1=bo_sb,
                                op0=Alu.mult,
                                op1=Alu.add,
                            )
                        # per-slice output DMAs: the last slice's transfer is
                        # small enough to barely extend the tail
                        nc.sync.dma_start(
                            out=out_r[qt, :, sb : sb + 1], in_=o_q[:, sb : sb + 1]
                        )

    nc.compile()
    return nc


def _emit_pv(nc, v_p, ones128, pt, ot, cs, kb, dcs):
    nc.tensor.matmul(
        cs, lhsT=ones128, rhs=pt, start=(kb == 0), stop=(kb == NB - 1)
    )
    for i, dc in enumerate(dcs):
        nc.tensor.matmul(
            ot[:, i, :],
            lhsT=v_p[kb // SB][:, kb % SB, dc * P : (dc + 1) * P],
            rhs=pt,
            start=(kb == 0),
            stop=(kb == NB - 1),
        )


def _get_compiled():
    global _compiled
    if _compiled is None:
        _compiled = _build()
    return _compiled


def _pack(x_t, nchunks):
    # [nchunks*P, S] -> [P, nchunks*S]: partition p holds chunks contiguously,
    # matching the SBUF tile layout exactly (max-size DMA packets)
    n = x_t.shape[1]
    return np.ascontiguousarray(
        x_t.reshape(nchunks, P, n).transpose(1, 0, 2).reshape(P, nchunks * n)
    )


def _host_tabs():
    half = D // 2
    inv_freq = 1.0 / (10000.0 ** (np.arange(half, dtype=np.float32) / half))
    t = np.arange(S, dtype=np.float32)
    freqs = np.outer(t, inv_freq)
    emb = np.concatenate([freqs, freqs], axis=-1)  # [S, D]
    # the two d-halves of emb are identical - ship only [D/2, S] worth, packed
    # slice-major with cos/sin and both pair-chunks interleaved per slice
    cos_h = np.cos(emb).T[:half].astype(BF16)  # [256, S]
    sin_h = np.sin(emb).T[:half].astype(BF16)
    tabs = np.empty((P, NS, 2, 2, QW), dtype=BF16)
    for pair in range(2):
        tabs[:, :, 0, pair, :] = cos_h[pair * P : (pair + 1) * P].reshape(P, NS, QW)
        tabs[:, :, 1, pair, :] = sin_h[pair * P : (pair + 1) * P].reshape(P, NS, QW)
    return np.ascontiguousarray(tabs.reshape(P, NS * 2 * 2 * QW))


def make_in_maps(**inputs):
    bkq = np.stack(
        [
            np.asarray(inputs["bk"], np.float32).reshape(DC, P).T,
            np.asarray(inputs["bq"], np.float32).reshape(DC, P).T,
        ],
        axis=1,
    )  # [P, 2, DC]
    shared = {
        "tabs": _host_tabs(),
        "wq_t": _pack(np.asarray(inputs["Wq"], np.float32).T.astype(BF16), EC),
        "wk_t": _pack(np.asarray(inputs["Wk"], np.float32).T.astype(BF16), EC),
        "wv_t": _pack(np.asarray(inputs["Wv"], np.float32).T.astype(BF16), EC),
        "wo_t": _pack(np.asarray(inputs["Wo"], np.float32).T.astype(BF16), EC),
        "bkq_c": np.ascontiguousarray(bkq),
        # bv contributes bv @ Wo.T to every output row - fold it into bo
        "bo_b": np.ascontiguousarray(
            np.broadcast_to(
                np.asarray(inputs["bo"], np.float32)
                + np.asarray(inputs["Wo"], np.float32)
                @ np.asarray(inputs["bv"], np.float32),
                (P, D),
            )
        ),
    }
    h1 = np.asarray(inputs["h1"], np.float32)
    h2 = np.asarray(inputs["h2"], np.float32)

    def _pack_h(h):
        # [S, D] -> [P, NS, EC, QW]: t[p, s2, ec, sq] = h[s2*QW+sq, ec*P+p]
        ht = h.T.astype(BF16)  # [D, S]
        return np.ascontiguousarray(
            ht.reshape(EC, P, NS, QW).transpose(1, 2, 0, 3)
        )

    return [
        dict(shared, h1t=_pack_h(h1[core]), h2t=_pack_h(h2[core]))
        for core in range(B)
    ]


def _install_ntff_hook():
    """The agent image's antenv lacks axon_hooks; rebuild the NTFF profile hook
    from libaxon_pjrt.so (mirrors trn_agent_boot._ntff_profile_via_ctypes)."""
    try:
        from antenv.axon_hooks import get_axon_ntff_profile_hook  # noqa: F401

        return
    except ImportError:
        pass
    import contextlib
    import ctypes
    import types

    so_path = "/opt/axon/libaxon_pjrt.so"
    try:
        lib = ctypes.CDLL(so_path)
    except OSError:
        return
    if not hasattr(lib, "axon_start_nrt_profile"):
        return
    lib.axon_start_nrt_profile.argtypes = [
        ctypes.POINTER(ctypes.c_int64),
        ctypes.c_size_t,
    ]
    lib.axon_start_nrt_profile.restype = ctypes.c_int64
    lib.axon_stop_nrt_profile.argtypes = [ctypes.c_char_p]
    lib.axon_stop_nrt_profile.restype = ctypes.c_int64

    @contextlib.contextmanager
    def _hook(output_dir, device_ids):
        import jax

        jax.devices()
        if device_ids:
            ids = (ctypes.c_int64 * len(device_ids))(*device_ids)
            rc = lib.axon_start_nrt_profile(ids, len(device_ids))
        else:
            rc = lib.axon_start_nrt_profile(None, 0)
        if rc != 0:
            raise RuntimeError(f"axon_start_nrt_profile rc={rc}")
        try:
            yield
        finally:
            n = lib.axon_stop_nrt_profile(str(output_dir).encode())
            print(f"ntff profile: {n} file(s) written to {output_dir}")

    import antenv

    mod = types.ModuleType("antenv.axon_hooks")
    mod.get_axon_ntff_profile_hook = lambda: _hook
    mod.set_axon_ntff_profile_hook = lambda h: None
    sys.modules["antenv.axon_hooks"] = mod
    antenv.axon_hooks = mod


def run(trace=False, tmpdir=None, trace_cores=None, **inputs):
    from concourse.bass_utils import run_bass_kernel_spmd

    if trace:
        _install_ntff_hook()
    nc = _get_compiled()
    in_maps = make_in_maps(**inputs)
    kwargs = {}
    if tmpdir is not None:
        kwargs["tmpdir"] = tmpdir
    if trace_cores is not None:
        kwargs["trace_cores"] = trace_cores
    res = run_bass_kernel_spmd(
        nc, in_maps, core_ids=list(range(B)), trace=trace, **kwargs
    )
    out = np.stack([res.results[i]["out"] for i in range(B)]).astype(np.float32)
    return out, res


def kernel(**inputs):
    out, _ = run(trace=False, **inputs)
    return out
